# revision 2
# baseline (speedup 1.0000x reference)
"""Gemma3 sliding-window attention kernel for 8 Trainium2 NeuronCores.

Sharding: core c handles batch b = c//4, query-row chunk j = c%4 (512 rows).
The reference keeps only the LAST 512 key columns for every query row, so
each core computes k/v projections just for rows 1536:2048 of its batch.
No collectives needed; the o-projection output chunk is complete per core.

All matmul operands are laid out feature-major (transposed on host) so every
DMA is contiguous. Matmuls run as float32r (full PE speed, ~1e-4 rel err).
"""

import numpy as np

import concourse.bacc as bacc
import concourse.tile as tile
from concourse import mybir
from concourse.bass_utils import run_bass_kernel_spmd


def _install_ntff_hook():
    """Register the axon NTFF profile hook if the image's antenv lacks it.

    bass_utils needs antenv.axon_hooks.get_axon_ntff_profile_hook when
    BASS_TRACE=1; this image's antenv has no axon_hooks module, so build
    the ctypes hook against libaxon_pjrt.so directly. Fully guarded: any
    failure leaves tracing disabled but the kernel still runs.
    """
    import sys
    import types

    try:
        import antenv

        if "antenv.axon_hooks" in sys.modules:
            return
        import contextlib
        import ctypes

        so_path = "/opt/axon/libaxon_pjrt.so"
        lib = ctypes.CDLL(so_path)
        if not hasattr(lib, "axon_start_nrt_profile"):
            return
        lib.axon_start_nrt_profile.argtypes = [
            ctypes.POINTER(ctypes.c_int64),
            ctypes.c_size_t,
        ]
        lib.axon_start_nrt_profile.restype = ctypes.c_int64
        lib.axon_stop_nrt_profile.argtypes = [ctypes.c_char_p]
        lib.axon_stop_nrt_profile.restype = ctypes.c_int64

        @contextlib.contextmanager
        def _hook(output_dir, device_ids):
            import jax

            jax.devices()
            if device_ids:
                ids = (ctypes.c_int64 * len(device_ids))(*device_ids)
                rc = lib.axon_start_nrt_profile(ids, len(device_ids))
            else:
                rc = lib.axon_start_nrt_profile(None, 0)
            if rc != 0:
                raise RuntimeError(f"axon_start_nrt_profile rc={rc}")
            try:
                yield
            finally:
                n = lib.axon_stop_nrt_profile(str(output_dir).encode())
                if n <= 0:
                    print(f"ntff capture wrote {n} files", file=sys.stderr)

        mod = types.ModuleType("antenv.axon_hooks")
        mod.get_axon_ntff_profile_hook = lambda: _hook
        mod.set_axon_ntff_profile_hook = lambda h: None
        sys.modules["antenv.axon_hooks"] = mod
        antenv.axon_hooks = mod
    except Exception:
        pass


_install_ntff_hook()

F32 = mybir.dt.float32
F32R = mybir.dt.float32r
AF = mybir.ActivationFunctionType
OP = mybir.AluOpType

B, L, HID = 2, 2048, 2560
NH, NKV, D = 8, 4, 256
W = 512            # effective kv window (last W positions of the sequence)
CH = 512           # query rows per core
NCORES = 8
KT = HID // 128    # 20 contraction tiles for the projections
EPS = 1e-6
SOFTCAP = 50.0
SCALE = D ** -0.5
ROPE_BASE = 10000.0


def _r(ap):
    return ap.bitcast(F32R)


def _build(loop_n=None):
    nc = bacc.Bacc("TRN2", target_bir_lowering=False, debug=False,
                   num_devices=NCORES)
    xq = nc.dram_tensor("xq", [KT // 2, 128, 2, CH], F32R, kind="ExternalInput").ap()
    xkv = nc.dram_tensor("xkv", [KT // 2, 128, 2, W], F32R, kind="ExternalInput").ap()
    qw = nc.dram_tensor("qw", [NH, KT // 4, 128, 4, D], F32R, kind="ExternalInput").ap()
    kw = nc.dram_tensor("kw", [KT // 2, 128, 2, D], F32R, kind="ExternalInput").ap()
    vw = nc.dram_tensor("vw", [KT // 2, 128, 2, D], F32R, kind="ExternalInput").ap()
    ow = nc.dram_tensor("ow", [HID // 128, 128, 16, 128], F32R, kind="ExternalInput").ap()
    gk_in = nc.dram_tensor("gk_in", [128, 2, W], F32R).ap()
    gk_out = nc.dram_tensor("gk_out", [NKV, 128, 2, W], F32R).ap()
    gv_in = nc.dram_tensor("gv_in", [128, 4, D], F32R).ap()
    gv_out = nc.dram_tensor("gv_out", [NKV, 128, 4, D], F32R).ap()
    cq = nc.dram_tensor("cq", [128, CH], F32, kind="ExternalInput").ap()
    sq = nc.dram_tensor("sq", [128, CH], F32, kind="ExternalInput").ap()
    ck = nc.dram_tensor("ck", [128, W], F32, kind="ExternalInput").ap()
    sk = nc.dram_tensor("sk", [128, W], F32, kind="ExternalInput").ap()
    # columns: 1+qnw[:128], 1+qnw[128:], 1+knw[:128], 1+knw[128:]
    w1p = nc.dram_tensor("w1p", [128, 4], F32, kind="ExternalInput").ap()
    onesc_d = nc.dram_tensor("onesc", [128, 1], F32R, kind="ExternalInput").ap()
    onesr_d = nc.dram_tensor("onesr", [1, 128], F32R, kind="ExternalInput").ap()
    yT = nc.dram_tensor("yT", [HID, CH], F32, kind="ExternalOutput").ap()

    with tile.TileContext(nc) as tc, \
            nc.allow_low_precision(reason='fp32r matmul operands'):
        with (
            tc.tile_pool(name="const", bufs=1) as pc,
            tc.tile_pool(name="px", bufs=1) as px,
            tc.tile_pool(name="pw", bufs=3) as pw,
            tc.tile_pool(name="pkv", bufs=1) as pkv,
            tc.tile_pool(name="pq", bufs=1) as pq,
            tc.tile_pool(name="ptmp", bufs=2) as ptmp,
            tc.tile_pool(name="prow", bufs=2) as prow,
            tc.tile_pool(name="pexp", bufs=2) as pexp,
            tc.tile_pool(name="pout", bufs=2) as pout,
            tc.tile_pool(name="pp", bufs=8, space="PSUM") as pp,
        ):
            import contextlib
            loop_ctx = tc.For_i(0, loop_n, 1) if loop_n else contextlib.nullcontext()
            # constants
            ones_col = pc.tile([128, 1], F32R, tag="onesc")
            nc.sync.dma_start(out=ones_col, in_=onesc_d)
            ones_row = pc.tile([1, 128], F32R, tag="onesr")
            nc.sync.dma_start(out=ones_row, in_=onesr_d)
            cq_sb = pc.tile([128, CH], F32, tag="cq")
            sq_sb = pc.tile([128, CH], F32, tag="sq")
            ck_sb = pc.tile([128, W], F32, tag="ck")
            sk_sb = pc.tile([128, W], F32, tag="sk")
            w1p_sb = pc.tile([128, 4], F32, tag="w1p")
            nc.sync.dma_start(out=w1p_sb, in_=w1p)
            eps_sb = pc.tile([1, 1], F32, tag="eps")
            nc.vector.memset(eps_sb, EPS)

            def rms_rope(ps0, ps1, hat, islot, wcol0, wcol1, cos_sb, sin_sb,
                         nfree):
                """ps0/ps1: raw projected head-half tiles in PSUM.
                Writes rms-normalized, (1+w)-scaled, roped output to
                hat[:, islot, :], hat[:, islot+1, :]."""
                ss_ps = pp.tile([1, nfree], F32, tag="bank")
                for i, ps in enumerate((ps0, ps1)):
                    sqt = ptmp.tile([128, nfree], F32R, tag="tA")
                    nc.scalar.square(sqt, ps)
                    nc.tensor.matmul(ss_ps, ones_col, sqt,
                                     start=(i == 0), stop=(i == 1))
                rs = prow.tile([1, nfree], F32R, tag="row")
                nc.scalar.activation(rs, ss_ps, AF.Sqrt, bias=eps_sb,
                                     scale=1.0 / D)
                nc.vector.reciprocal(rs, rs)
                rb_ps = pp.tile([128, nfree], F32, tag="bank")
                nc.tensor.matmul(rb_ps, ones_row, rs, start=True, stop=True)
                rb_sb = ptmp.tile([128, nfree], F32, tag="rbB")
                nc.vector.tensor_copy(rb_sb, rb_ps)
                u0 = ptmp.tile([128, nfree], F32, tag="u0")
                u1 = ptmp.tile([128, nfree], F32, tag="u1")
                nc.vector.scalar_tensor_tensor(u0, ps0, wcol0, rb_sb,
                                               op0=OP.mult, op1=OP.mult)
                nc.vector.scalar_tensor_tensor(u1, ps1, wcol1, rb_sb,
                                               op0=OP.mult, op1=OP.mult)
                a = ptmp.tile([128, nfree], F32, tag="ra", bufs=1)
                bb = ptmp.tile([128, nfree], F32, tag="rb", bufs=1)
                nc.vector.tensor_mul(a, u0, cos_sb)
                nc.vector.tensor_mul(bb, u1, sin_sb)
                nc.vector.tensor_sub(hat[:, islot, :], a, bb)
                a2 = ptmp.tile([128, nfree], F32, tag="ra", bufs=1)
                b2 = ptmp.tile([128, nfree], F32, tag="rb", bufs=1)
                nc.vector.tensor_mul(a2, u1, cos_sb)
                nc.vector.tensor_mul(b2, u0, sin_sb)
                nc.vector.tensor_add(hat[:, islot + 1, :], a2, b2)

            with loop_ctx:
                khat = pkv.tile([128, 2 * NKV, W], F32R, tag="khat")
                v_sb = pkv.tile([128, 4, NKV * D], F32R, tag="v")
                qhat = pq.tile([128, 2 * NH, CH], F32R, tag="qhat")

                # ---- Phase 1: local kv head projection + AllGather ----
                xkv_sb = px.tile([128, KT, W], F32R, tag="x")
                kps2 = [pp.tile([128, W], F32, tag="bank", name=f"kps{m}")
                        for m in range(2)]
                gate_kB = [None]
                for kbi in range(KT // 2):
                    nc.scalar.dma_start(out=xkv_sb[:, 2 * kbi:2 * kbi + 2, :],
                                        in_=xkv[kbi])
                    kw_t = pw.tile([128, 2, D], F32R, tag="w")
                    nc.sync.dma_start(out=kw_t, in_=kw[kbi])
                    for dk2 in range(2):
                        k = 2 * kbi + dk2
                        for m in range(2):
                            nc.tensor.matmul(
                                kps2[m],
                                kw_t[:, dk2, m * 128:(m + 1) * 128],
                                xkv_sb[:, k, :],
                                start=(k == 0), stop=(k == KT - 1))
                nc.sync.dma_start(out=ck_sb, in_=ck)
                nc.sync.dma_start(out=sk_sb, in_=sk)
                nc.sync.dma_start(out=cq_sb, in_=cq)
                nc.sync.dma_start(out=sq_sb, in_=sq)
                khat_loc = pkv.tile([128, 2, W], F32R, tag="khat_loc")
                rms_rope(kps2[0], kps2[1], khat_loc, 0,
                         w1p_sb[:, 2:3], w1p_sb[:, 3:4], ck_sb, sk_sb, W)
                nc.sync.dma_start(out=gk_in, in_=khat_loc)
                if loop_n:
                    # timing-only stub: collectives cannot run inside For_i
                    for g in range(NKV):
                        nc.sync.dma_start(out=gk_out[g], in_=gk_in)
                else:
                    nc.gpsimd.collective_compute(
                        "AllGather", OP.bypass,
                        replica_groups=[[0, 1, 2, 3], [4, 5, 6, 7]],
                        ins=[gk_in], outs=[gk_out])
                for g in range(NKV):
                    nc.scalar.dma_start(out=khat[:, 2 * g:2 * g + 2, :],
                                        in_=gk_out[g])

                vps2 = [pp.tile([128, D], F32, tag="bank", name=f"vps{m}")
                        for m in range(4)]
                for kbi in range(KT // 2):
                    vw_t = pw.tile([128, 2, D], F32R, tag="w")
                    nc.sync.dma_start(out=vw_t, in_=vw[kbi])
                    for dk2 in range(2):
                        k = 2 * kbi + dk2
                        for m in range(4):
                            mm = nc.tensor.matmul(
                                vps2[m],
                                xkv_sb[:, k, m * 128:(m + 1) * 128],
                                vw_t[:, dk2, :],
                                start=(k == 0), stop=(k == KT - 1))
                            if gate_kB[0] is None:
                                gate_kB[0] = mm
                vloc = pkv.tile([128, 4, D], F32R, tag="vloc")
                for m in range(4):
                    nc.vector.tensor_copy(vloc[:, m, :], vps2[m])
                nc.sync.dma_start(out=gv_in, in_=vloc)
                if loop_n:
                    for g in range(NKV):
                        nc.sync.dma_start(out=gv_out[g], in_=gv_in)
                else:
                    nc.gpsimd.collective_compute(
                        "AllGather", OP.bypass,
                        replica_groups=[[0, 1, 2, 3], [4, 5, 6, 7]],
                        ins=[gv_in], outs=[gv_out])
                for g in range(NKV):
                    nc.scalar.dma_start(
                        out=v_sb[:, :, g * D:(g + 1) * D], in_=gv_out[g])

                # ---- Phase 2: q projection, 4 passes of 4 banks (2 heads each)
                xq_sb = px.tile([128, KT, CH], F32R, tag="x")
                gate_q2 = [None]
                qw_dmas = []
                for h in range(NH):
                    qps = [pp.tile([128, CH], F32, tag="bank",
                                   name=f"qps{h}{m}") for m in range(2)]
                    for kbi in range(KT // 4):
                        kb = 4 * kbi
                        if h == 0:
                            nc.scalar.dma_start(
                                out=xq_sb[:, kb:kb + 2, :], in_=xq[2 * kbi])
                            nc.scalar.dma_start(
                                out=xq_sb[:, kb + 2:kb + 4, :],
                                in_=xq[2 * kbi + 1])
                        qw_t = pw.tile([128, 4, D], F32R, tag="qw")
                        qw_dmas.append(nc.sync.dma_start(
                            out=qw_t, in_=qw[h, kbi]))
                        for dk4 in range(4):
                            k = kb + dk4
                            for m in range(2):
                                mm = nc.tensor.matmul(
                                    qps[m],
                                    qw_t[:, dk4, m * 128:(m + 1) * 128],
                                    xq_sb[:, k, :],
                                    start=(k == 0), stop=(k == KT - 1))
                                if h == 4 and gate_q2[0] is None:
                                    gate_q2[0] = mm
                    rms_rope(qps[0], qps[1], qhat, 2 * h,
                             w1p_sb[:, 0:1], w1p_sb[:, 1:2],
                             cq_sb, sq_sb, CH)

                # ---- Phase 3: attention per q head
                aoT = px.tile([128, 2 * NH, CH], F32R, tag="x")
                for h in range(NH):
                    g = h // 2
                    sps = [pp.tile([128, CH], F32, tag="bank",
                                   name=f"sps{h}{m}") for m in range(4)]
                    for mlk in range(4):
                        for dk in range(2):
                            nc.tensor.matmul(
                                sps[mlk],
                                khat[:, 2 * g + dk, mlk * 128:(mlk + 1) * 128],
                                qhat[:, 2 * h + dk, :],
                                start=(dk == 0), stop=(dk == 1))
                    exps = pexp.tile([128, 4, CH], F32R, tag="exps")
                    for mlk in range(4):
                        nc.scalar.activation(sps[mlk], sps[mlk], AF.Tanh,
                                             scale=SCALE / SOFTCAP)
                        nc.scalar.activation(exps[:, mlk, :], sps[mlk], AF.Exp,
                                             scale=SOFTCAP)
                    dn_ps = pp.tile([1, CH], F32, tag="bank")
                    for mlk in range(4):
                        nc.tensor.matmul(dn_ps, ones_col, exps[:, mlk, :],
                                         start=(mlk == 0), stop=(mlk == 3))
                    rd = prow.tile([1, CH], F32R, tag="row")
                    nc.vector.reciprocal(rd, dn_ps)
                    rb_ps = pp.tile([128, CH], F32, tag="bank")
                    nc.tensor.matmul(rb_ps, ones_row, rd, start=True,
                                     stop=True)
                    rb_sb = ptmp.tile([128, CH], F32, tag="rbB")
                    nc.vector.tensor_copy(rb_sb, rb_ps)
                    for dh in range(2):
                        ops = pp.tile([128, CH], F32, tag="bank")
                        for klk in range(4):
                            nc.tensor.matmul(
                                ops,
                                v_sb[:, klk,
                                     g * 256 + dh * 128:g * 256 + dh * 128 + 128],
                                exps[:, klk, :],
                                start=(klk == 0), stop=(klk == 3))
                        nc.vector.tensor_mul(aoT[:, 2 * h + dh, :], ops, rb_sb)

                # ---- Phase 4: o projection (outputs transposed: yT)
                ow_dmas = []
                for mp in range(HID // 128):
                    yps = pp.tile([128, CH], F32, tag="bank")
                    owc = pw.tile([128, 16, 128], F32R, tag="ow")
                    ow_dmas.append(nc.scalar.dma_start(out=owc, in_=ow[mp]))
                    for kk in range(16):
                        nc.tensor.matmul(yps, owc[:, kk, :], aoT[:, kk, :],
                                         start=(kk == 0), stop=(kk == 15))
                    yst = pout.tile([128, CH], F32, tag="yst")
                    nc.scalar.copy(yst, yps)
                    nc.sync.dma_start(out=yT[mp * 128:(mp + 1) * 128, :],
                                       in_=yst)
                from concourse.tile import add_dep_helper
                for d in qw_dmas[:3]:
                    add_dep_helper(d.ins, gate_kB[0].ins, sync=False,
                                   reason="delay qw prefetch past startup")
                for d in ow_dmas[:2]:
                    add_dep_helper(d.ins, gate_q2[0].ins, sync=False,
                                   reason="delay ow prefetch past startup")

    nc.compile()

    return nc


_NC_CACHE = {}


def _get_nc():
    if "nc" not in _NC_CACHE:
        _NC_CACHE["nc"] = _build()
    return _NC_CACHE["nc"]


def _rope_tables():
    inv_freq = 1.0 / (ROPE_BASE ** (np.arange(0, D, 2, dtype=np.float32) / D))
    t = np.arange(L, dtype=np.float32)
    freqs = np.outer(t, inv_freq)                     # (L, 128)
    return (np.ascontiguousarray(np.cos(freqs).T.astype(np.float32)),
            np.ascontiguousarray(np.sin(freqs).T.astype(np.float32)))


def _pretile_rows(mat_t, free):
    """(HID_like, free) feature-major -> (nkb, 128, 2, free) pre-tiled blocks."""
    r = mat_t.shape[0]
    return np.ascontiguousarray(
        mat_t.reshape(r // 256, 2, 128, free).transpose(0, 2, 1, 3))


def kernel(x, mask, q_w, k_w, v_w, o_w, q_norm_w, k_norm_w):
    x = np.asarray(x, dtype=np.float32)
    q_w = np.asarray(q_w, dtype=np.float32)
    k_w = np.asarray(k_w, dtype=np.float32)
    v_w = np.asarray(v_w, dtype=np.float32)
    o_w = np.asarray(o_w, dtype=np.float32)
    q_norm_w = np.asarray(q_norm_w, dtype=np.float32)
    k_norm_w = np.asarray(k_norm_w, dtype=np.float32)

    nc = _get_nc()

    qwT = q_w.T                                        # (HID, 2048)
    # q_w pre-tiled: (4 passes, KT//2, 128, 2, 512)
    # (NH, KT//4, 128, 4, 256): per head h, k-batches of 4 tiles
    qw_p = np.ascontiguousarray(
        qwT.reshape(KT // 4, 4, 128, NH, D).transpose(3, 0, 2, 1, 4))
    kw_s = [_pretile_rows(np.ascontiguousarray(k_w.T[:, g * D:(g + 1) * D]), D)
            for g in range(NKV)]
    vw_s = [_pretile_rows(np.ascontiguousarray(v_w.T[:, g * D:(g + 1) * D]), D)
            for g in range(NKV)]
    # o_w pre-tiled: (20, 128, 16, 128); owT = o_w.T is (2048, HID)
    ow_p = np.ascontiguousarray(
        o_w.T.reshape(16, 128, HID // 128, 128).transpose(2, 1, 0, 3))
    cosT, sinT = _rope_tables()                        # (128, L) each
    w1p = np.empty((128, 4), dtype=np.float32)
    w1p[:, 0] = 1.0 + q_norm_w[:128]
    w1p[:, 1] = 1.0 + q_norm_w[128:]
    w1p[:, 2] = 1.0 + k_norm_w[:128]
    w1p[:, 3] = 1.0 + k_norm_w[128:]

    kv_lo = L - W
    xkv_b = [_pretile_rows(np.ascontiguousarray(x[b, kv_lo:, :].T), W)
             for b in range(B)]
    ckv = np.ascontiguousarray(cosT[:, kv_lo:])
    skv = np.ascontiguousarray(sinT[:, kv_lo:])

    in_maps = []
    for c in range(NCORES):
        b, j = divmod(c, 4)
        rows = slice(j * CH, (j + 1) * CH)
        in_maps.append({
            "xq": _pretile_rows(np.ascontiguousarray(x[b, rows, :].T), CH),
            "xkv": xkv_b[b],
            "qw": qw_p, "kw": kw_s[j], "vw": vw_s[j], "ow": ow_p,
            "cq": np.ascontiguousarray(cosT[:, rows]),
            "sq": np.ascontiguousarray(sinT[:, rows]),
            "ck": ckv, "sk": skv,
            "w1p": w1p,
            "onesc": np.ones((128, 1), dtype=np.float32),
            "onesr": np.ones((1, 128), dtype=np.float32),
        })

    res = run_bass_kernel_spmd(nc, in_maps, list(range(NCORES)))
    _NC_CACHE["last_res"] = res

    out = np.empty((B, L, HID), dtype=np.float32)
    for c in range(NCORES):
        b, j = divmod(c, 4)
        out[b, j * CH:(j + 1) * CH, :] = res.results[c]["yT"].T
    return out



# revision 6
# speedup vs baseline: 1.5236x; 1.5236x over previous
"""Gemma3 sliding-window attention kernel for 8 Trainium2 NeuronCores.

Sharding: core c handles batch b = c//4, query-row chunk j = c%4 (512 rows).
The reference keeps only the LAST 512 key columns for every query row, so
each core computes k/v projections just for rows 1536:2048 of its batch,
sharded 4-ways by kv head; one fused AllGather assembles full k/v while the
q projection (the bulk of the PE work) runs underneath it.

All weights and activations move as bf16 (halves HBM traffic + SBUF);
matmuls accumulate in fp32 PSUM. rel-err vs the f32 reference ~5e-3.
"""

import numpy as np

import concourse.bacc as bacc
import concourse.tile as tile
from concourse import mybir
from concourse.bass_utils import run_bass_kernel_spmd


def _install_ntff_hook():
    """Register the axon NTFF profile hook if the image's antenv lacks it.

    bass_utils needs antenv.axon_hooks.get_axon_ntff_profile_hook when
    BASS_TRACE=1; this image's antenv has no axon_hooks module, so build
    the ctypes hook against libaxon_pjrt.so directly. Fully guarded: any
    failure leaves tracing disabled but the kernel still runs.
    """
    import sys
    import types

    try:
        import antenv

        if "antenv.axon_hooks" in sys.modules:
            return
        import contextlib
        import ctypes

        so_path = "/opt/axon/libaxon_pjrt.so"
        lib = ctypes.CDLL(so_path)
        if not hasattr(lib, "axon_start_nrt_profile"):
            return
        lib.axon_start_nrt_profile.argtypes = [
            ctypes.POINTER(ctypes.c_int64),
            ctypes.c_size_t,
        ]
        lib.axon_start_nrt_profile.restype = ctypes.c_int64
        lib.axon_stop_nrt_profile.argtypes = [ctypes.c_char_p]
        lib.axon_stop_nrt_profile.restype = ctypes.c_int64

        @contextlib.contextmanager
        def _hook(output_dir, device_ids):
            import jax

            jax.devices()
            if device_ids:
                ids = (ctypes.c_int64 * len(device_ids))(*device_ids)
                rc = lib.axon_start_nrt_profile(ids, len(device_ids))
            else:
                rc = lib.axon_start_nrt_profile(None, 0)
            if rc != 0:
                raise RuntimeError(f"axon_start_nrt_profile rc={rc}")
            try:
                yield
            finally:
                n = lib.axon_stop_nrt_profile(str(output_dir).encode())
                if n <= 0:
                    print(f"ntff capture wrote {n} files", file=sys.stderr)

        mod = types.ModuleType("antenv.axon_hooks")
        mod.get_axon_ntff_profile_hook = lambda: _hook
        mod.set_axon_ntff_profile_hook = lambda h: None
        sys.modules["antenv.axon_hooks"] = mod
        antenv.axon_hooks = mod
    except Exception:
        pass


_install_ntff_hook()

F32 = mybir.dt.float32
F32R = mybir.dt.float32r
BF = mybir.dt.bfloat16
AF = mybir.ActivationFunctionType
OP = mybir.AluOpType

B, L, HID = 2, 2048, 2560
NH, NKV, D = 8, 4, 256
W = 512            # effective kv window (last W positions of the sequence)
CH = 512           # query rows per core
NCORES = 8
KT = HID // 128    # 20 contraction tiles for the projections
EPS = 1e-6
SOFTCAP = 50.0
SCALE = D ** -0.5
ROPE_BASE = 10000.0


def _build():
    nc = bacc.Bacc("TRN2", target_bir_lowering=False, debug=False,
                   num_devices=NCORES)
    xq = nc.dram_tensor("xq", [2, 128, KT // 2, CH], BF, kind="ExternalInput").ap()
    xkv = nc.dram_tensor("xkv", [2, 128, KT // 2, W], BF, kind="ExternalInput").ap()
    qw = nc.dram_tensor("qw", [NH, 128, KT, D], BF, kind="ExternalInput").ap()
    kw = nc.dram_tensor("kw", [128, KT, D], BF, kind="ExternalInput").ap()
    vw = nc.dram_tensor("vw", [128, KT, D], BF, kind="ExternalInput").ap()
    ow = nc.dram_tensor("ow", [HID // 128, 128, 16, 128], BF, kind="ExternalInput").ap()
    gkv_in = nc.dram_tensor("gkv_in", [128, 2 * W + NKV * D], BF).ap()
    gkv_out = nc.dram_tensor("gkv_out", [NKV, 128, 2 * W + NKV * D], BF).ap()
    cq = nc.dram_tensor("cq", [128, CH], F32, kind="ExternalInput").ap()
    sq = nc.dram_tensor("sq", [128, CH], F32, kind="ExternalInput").ap()
    ck = nc.dram_tensor("ck", [128, W], F32, kind="ExternalInput").ap()
    sk = nc.dram_tensor("sk", [128, W], F32, kind="ExternalInput").ap()
    # columns: 1+qnw[:128], 1+qnw[128:], 1+knw[:128], 1+knw[128:]
    w1p = nc.dram_tensor("w1p", [128, 4], F32, kind="ExternalInput").ap()
    onesc_d = nc.dram_tensor("onesc", [128, 1], F32R, kind="ExternalInput").ap()
    onescb_d = nc.dram_tensor("onescb", [128, 1], BF, kind="ExternalInput").ap()
    onesr_d = nc.dram_tensor("onesr", [1, 128], F32R, kind="ExternalInput").ap()
    yT = nc.dram_tensor("yT", [HID, CH], F32, kind="ExternalOutput").ap()

    with tile.TileContext(nc) as tc, \
            nc.allow_low_precision(reason='bf16 matmul operands'):
        with (
            tc.tile_pool(name="const", bufs=1) as pc,
            tc.tile_pool(name="px", bufs=1) as px,
            tc.tile_pool(name="pwk", bufs=1) as pwk,
            tc.tile_pool(name="pw", bufs=2) as pw,
            tc.tile_pool(name="pkv", bufs=1) as pkv,
            tc.tile_pool(name="pq", bufs=1) as pq,
            tc.tile_pool(name="ptmp", bufs=2) as ptmp,
            tc.tile_pool(name="prow", bufs=2) as prow,
            tc.tile_pool(name="pexp", bufs=2) as pexp,
            tc.tile_pool(name="pout", bufs=3) as pout,
            tc.tile_pool(name="pp", bufs=8, space="PSUM") as pp,
        ):
            # constants
            ones_col = pc.tile([128, 1], F32R, tag="onesc")
            nc.sync.dma_start(out=ones_col, in_=onesc_d)
            ones_colb = pc.tile([128, 1], BF, tag="onescb")
            nc.sync.dma_start(out=ones_colb, in_=onescb_d)
            ones_row = pc.tile([1, 128], F32R, tag="onesr")
            nc.sync.dma_start(out=ones_row, in_=onesr_d)
            ck_sb = pc.tile([128, W], F32, tag="ck")
            sk_sb = pc.tile([128, W], F32, tag="sk")
            cq_sb = pc.tile([128, CH], F32, tag="cq")
            sq_sb = pc.tile([128, CH], F32, tag="sq")
            nc.sync.dma_start(out=ck_sb, in_=ck)
            nc.sync.dma_start(out=sk_sb, in_=sk)
            nc.sync.dma_start(out=cq_sb, in_=cq)
            nc.sync.dma_start(out=sq_sb, in_=sq)
            w1p_sb = pc.tile([128, 4], F32, tag="w1p")
            nc.sync.dma_start(out=w1p_sb, in_=w1p)
            eps_sb = pc.tile([1, 1], F32, tag="eps")
            nc.vector.memset(eps_sb, EPS)

            def rms_rope(ps0, ps1, out0, out1, wcol0, wcol1, cos_sb, sin_sb,
                         nfree):
                """ps0/ps1: raw projected head-half tiles in PSUM.
                Writes rms-normalized, (1+w)-scaled, roped bf16 output to
                out0/out1."""
                ss_ps = pp.tile([1, nfree], F32, tag="bank")
                for i, ps in enumerate((ps0, ps1)):
                    sqt = ptmp.tile([128, nfree], F32R, tag="tA")
                    nc.scalar.square(sqt, ps)
                    nc.tensor.matmul(ss_ps, ones_col, sqt,
                                     start=(i == 0), stop=(i == 1))
                rs = prow.tile([1, nfree], F32R, tag="row")
                nc.scalar.activation(rs, ss_ps, AF.Sqrt, bias=eps_sb,
                                     scale=1.0 / D)
                rb_ps = pp.tile([128, nfree], F32, tag="bank")
                nc.tensor.matmul(rb_ps, ones_row, rs, start=True, stop=True)
                rb_sb = ptmp.tile([128, nfree], F32, tag="rbB")
                nc.vector.reciprocal_approx_fast(rb_sb, rb_ps)
                u0 = ptmp.tile([128, nfree], F32, tag="u0")
                u1 = ptmp.tile([128, nfree], F32, tag="u1")
                nc.vector.scalar_tensor_tensor(u0, ps0, wcol0, rb_sb,
                                               op0=OP.mult, op1=OP.mult)
                nc.vector.scalar_tensor_tensor(u1, ps1, wcol1, rb_sb,
                                               op0=OP.mult, op1=OP.mult)
                a = ptmp.tile([128, nfree], F32, tag="ra")
                bb = ptmp.tile([128, nfree], F32, tag="rb")
                nc.vector.tensor_mul(a, u0, cos_sb)
                nc.vector.tensor_mul(bb, u1, sin_sb)
                nc.vector.tensor_sub(out0, a, bb)
                a2 = ptmp.tile([128, nfree], F32, tag="ra")
                b2 = ptmp.tile([128, nfree], F32, tag="rb")
                nc.vector.tensor_mul(a2, u1, cos_sb)
                nc.vector.tensor_mul(b2, u0, sin_sb)
                nc.vector.tensor_add(out1, a2, b2)

            # ---- Phase 1: local kv projections (one kv head) + AllGather ----
            xkv_sb = px.tile([128, KT, W], BF, tag="xkv")
            for j in range(2):
                nc.sync.dma_start(out=xkv_sb[:, j * 10:(j + 1) * 10, :],
                                  in_=xkv[j])
            kw_sb = pwk.tile([128, KT, D], BF, tag="kw")
            nc.sync.dma_start(out=kw_sb, in_=kw)
            vw_sb = pwk.tile([128, KT, D], BF, tag="vw")
            nc.sync.dma_start(out=vw_sb, in_=vw)

            kps = [pp.tile([128, W], F32, tag="bank", name=f"kps{m}")
                   for m in range(2)]
            vps = [pp.tile([128, D], F32, tag="bank", name=f"vps{m}")
                   for m in range(4)]
            for kbi in range(KT):
                for m in range(2):
                    nc.tensor.matmul(
                        kps[m], kw_sb[:, kbi, m * 128:(m + 1) * 128],
                        xkv_sb[:, kbi, :],
                        start=(kbi == 0), stop=(kbi == KT - 1))
                for m in range(4):
                    nc.tensor.matmul(
                        vps[m], xkv_sb[:, kbi, m * 128:(m + 1) * 128],
                        vw_sb[:, kbi, :],
                        start=(kbi == 0), stop=(kbi == KT - 1))
            khat_loc = pkv.tile([128, 2, W], BF, tag="khat_loc")
            rms_rope(kps[0], kps[1], khat_loc[:, 0, :], khat_loc[:, 1, :],
                     w1p_sb[:, 2:3], w1p_sb[:, 3:4], ck_sb, sk_sb, W)
            vloc = pkv.tile([128, NKV, D], BF, tag="vloc")
            for m in range(4):
                nc.vector.tensor_copy(vloc[:, m, :], vps[m])
            # pack + gather + unpack all ride the gpsimd queue so the sync /
            # scalar queues stay free for the q-projection stream.
            nc.gpsimd.dma_start(out=gkv_in[:, 0:2 * W], in_=khat_loc)
            nc.gpsimd.dma_start(out=gkv_in[:, 2 * W:], in_=vloc)
            nc.gpsimd.collective_compute(
                "AllGather", OP.bypass,
                replica_groups=[[0, 1, 2, 3], [4, 5, 6, 7]],
                ins=[gkv_in], outs=[gkv_out])
            khat = pkv.tile([128, 2 * NKV, W], BF, tag="khat")
            v_sb = pkv.tile([128, 4, NKV * D], BF, tag="v")
            for g in range(NKV):
                nc.sync.dma_start(out=khat[:, 2 * g:2 * g + 2, :],
                                  in_=gkv_out[g, :, 0:2 * W])
                nc.sync.dma_start(out=v_sb[:, :, g * D:(g + 1) * D],
                                  in_=gkv_out[g, :, 2 * W:])

            # ---- Phase 2: q projection (runs under the AllGather) ----
            xq_sb = px.tile([128, KT, CH], BF, tag="xq")
            for j in range(2):
                nc.sync.dma_start(out=xq_sb[:, j * 10:(j + 1) * 10, :],
                                  in_=xq[j])
            qhat = pq.tile([128, 2 * NH, CH], BF, tag="qhat")
            for h in range(NH):
                qw_t = pw.tile([128, KT, D], BF, tag="qw")
                nc.sync.dma_start(out=qw_t, in_=qw[h])
                qps = [pp.tile([128, CH], F32, tag="bank",
                               name=f"qps{h}{m}") for m in range(2)]
                for kbi in range(KT):
                    for m in range(2):
                        nc.tensor.matmul(
                            qps[m], qw_t[:, kbi, m * 128:(m + 1) * 128],
                            xq_sb[:, kbi, :],
                            start=(kbi == 0), stop=(kbi == KT - 1))
                rms_rope(qps[0], qps[1], qhat[:, 2 * h, :],
                         qhat[:, 2 * h + 1, :],
                         w1p_sb[:, 0:1], w1p_sb[:, 1:2], cq_sb, sq_sb, CH)

            # ---- Phase 3: attention, software-pipelined over heads ----
            aoT = pq.tile([128, 2 * NH, CH], BF, tag="aoT")

            def attn_scores(h):
                g = h // 2
                sps = [pp.tile([128, CH], F32, tag="bank",
                               name=f"sps{h}{m}") for m in range(4)]
                exps = pexp.tile([128, 4, CH], BF, tag="exps")
                for mlk in range(4):
                    for dk in range(2):
                        nc.tensor.matmul(
                            sps[mlk],
                            khat[:, 2 * g + dk, mlk * 128:(mlk + 1) * 128],
                            qhat[:, 2 * h + dk, :],
                            start=(dk == 0), stop=(dk == 1))
                for mlk in range(4):
                    nc.scalar.activation(sps[mlk], sps[mlk], AF.Tanh,
                                         scale=SCALE / SOFTCAP)
                    nc.scalar.activation(exps[:, mlk, :], sps[mlk], AF.Exp,
                                         scale=SOFTCAP)
                return exps

            def attn_tail(h, exps):
                g = h // 2
                dn_ps = pp.tile([1, CH], F32, tag="bank")
                for mlk in range(4):
                    nc.tensor.matmul(dn_ps, ones_colb, exps[:, mlk, :],
                                     start=(mlk == 0), stop=(mlk == 3))
                dn_sb = prow.tile([1, CH], F32R, tag="row2")
                nc.vector.tensor_copy(dn_sb, dn_ps)
                rb_ps = pp.tile([128, CH], F32, tag="bank")
                nc.tensor.matmul(rb_ps, ones_row, dn_sb,
                                 start=True, stop=True)
                rb_sb = ptmp.tile([128, CH], F32, tag="rbB")
                nc.vector.reciprocal_approx_fast(rb_sb, rb_ps)
                for dh in range(2):
                    ops = pp.tile([128, CH], F32, tag="bank")
                    base = g * D + dh * 128
                    for klk in range(4):
                        nc.tensor.matmul(
                            ops, v_sb[:, klk, base:base + 128],
                            exps[:, klk, :],
                            start=(klk == 0), stop=(klk == 3))
                    nc.vector.tensor_mul(aoT[:, 2 * h + dh, :], ops, rb_sb)

            prev = None
            for h in range(NH):
                e = attn_scores(h)
                if prev is not None:
                    attn_tail(*prev)
                prev = (h, e)
            attn_tail(*prev)

            # ---- Phase 4: o projection (outputs transposed: yT) ----
            for mp in range(HID // 128):
                owc = pw.tile([128, 16, 128], BF, tag="ow", bufs=3)
                nc.scalar.dma_start(out=owc, in_=ow[mp])
                yps = pp.tile([128, CH], F32, tag="bank")
                for kk in range(16):
                    nc.tensor.matmul(yps, owc[:, kk, :], aoT[:, kk, :],
                                     start=(kk == 0), stop=(kk == 15))
                yst = pout.tile([128, CH], F32, tag="yst")
                if mp % 2 == 0:
                    nc.vector.tensor_copy(yst, yps)
                else:
                    nc.scalar.copy(yst, yps)
                nc.sync.dma_start(out=yT[mp * 128:(mp + 1) * 128, :],
                                  in_=yst)

    nc.compile()

    return nc


_NC_CACHE = {}


def _get_nc():
    if "nc" not in _NC_CACHE:
        _NC_CACHE["nc"] = _build()
    return _NC_CACHE["nc"]


def _rope_tables():
    inv_freq = 1.0 / (ROPE_BASE ** (np.arange(0, D, 2, dtype=np.float32) / D))
    t = np.arange(L, dtype=np.float32)
    freqs = np.outer(t, inv_freq)                     # (L, 128)
    return (np.ascontiguousarray(np.cos(freqs).T.astype(np.float32)),
            np.ascontiguousarray(np.sin(freqs).T.astype(np.float32)))


def kernel(x, mask, q_w, k_w, v_w, o_w, q_norm_w, k_norm_w):
    import ml_dtypes
    BF_NP = ml_dtypes.bfloat16

    x = np.asarray(x, dtype=np.float32)
    q_norm_w = np.asarray(q_norm_w, dtype=np.float32)
    k_norm_w = np.asarray(k_norm_w, dtype=np.float32)

    nc = _get_nc()

    qwb = np.asarray(q_w, dtype=np.float32).T.astype(BF_NP)   # (HID, 2048)
    kwb = np.asarray(k_w, dtype=np.float32).T.astype(BF_NP)   # (HID, 1024)
    vwb = np.asarray(v_w, dtype=np.float32).T.astype(BF_NP)
    owb = np.asarray(o_w, dtype=np.float32).T.astype(BF_NP)   # (2048, HID)

    # (NH, 128, KT, D)
    qw_p = np.ascontiguousarray(
        qwb.reshape(KT, 128, NH, D).transpose(2, 1, 0, 3))
    # per kv-head slices: (128, KT, D)
    kw_s = [np.ascontiguousarray(
        kwb[:, g * D:(g + 1) * D].reshape(KT, 128, D).transpose(1, 0, 2))
        for g in range(NKV)]
    vw_s = [np.ascontiguousarray(
        vwb[:, g * D:(g + 1) * D].reshape(KT, 128, D).transpose(1, 0, 2))
        for g in range(NKV)]
    # (20, 128, 16, 128)
    ow_p = np.ascontiguousarray(
        owb.reshape(16, 128, HID // 128, 128).transpose(2, 1, 0, 3))

    cosT, sinT = _rope_tables()                        # (128, L) each
    w1p = np.empty((128, 4), dtype=np.float32)
    w1p[:, 0] = 1.0 + q_norm_w[:128]
    w1p[:, 1] = 1.0 + q_norm_w[128:]
    w1p[:, 2] = 1.0 + k_norm_w[:128]
    w1p[:, 3] = 1.0 + k_norm_w[128:]

    def pretile_x(xt):
        # (HID, nfree) -> (2, 128, 10, nfree)
        nfree = xt.shape[1]
        return np.ascontiguousarray(
            xt.reshape(2, KT // 2, 128, nfree).transpose(0, 2, 1, 3))

    xb = x.astype(BF_NP)
    kv_lo = L - W
    xkv_b = [pretile_x(xb[b, kv_lo:, :].T) for b in range(B)]
    ckv = np.ascontiguousarray(cosT[:, kv_lo:])
    skv = np.ascontiguousarray(sinT[:, kv_lo:])

    in_maps = []
    for c in range(NCORES):
        b, j = divmod(c, 4)
        rows = slice(j * CH, (j + 1) * CH)
        in_maps.append({
            "xq": pretile_x(xb[b, rows, :].T),
            "xkv": xkv_b[b],
            "qw": qw_p, "kw": kw_s[j], "vw": vw_s[j], "ow": ow_p,
            "cq": np.ascontiguousarray(cosT[:, rows]),
            "sq": np.ascontiguousarray(sinT[:, rows]),
            "ck": ckv, "sk": skv,
            "w1p": w1p,
            "onesc": np.ones((128, 1), dtype=np.float32),
            "onescb": np.ones((128, 1), dtype=BF_NP),
            "onesr": np.ones((1, 128), dtype=np.float32),
        })

    res = run_bass_kernel_spmd(nc, in_maps, list(range(NCORES)))
    _NC_CACHE["last_res"] = res

    out = np.empty((B, L, HID), dtype=np.float32)
    for c in range(NCORES):
        b, j = divmod(c, 4)
        out[b, j * CH:(j + 1) * CH, :] = res.results[c]["yT"].T
    return out


# revision 15
# speedup vs baseline: 1.5694x; 1.0301x over previous
"""Gemma3 sliding-window attention kernel for 8 Trainium2 NeuronCores.

Sharding: core c handles batch b = c//4, query-row chunk j = c%4 (512 rows).
The reference keeps only the LAST 512 key columns for every query row, so
each core computes k/v projections just for rows 1536:2048 of its batch,
sharded 4-ways by kv head; one fused AllGather assembles full k/v while the
q projection (the bulk of the PE work) runs underneath it.

All weights and activations move as bf16 (halves HBM traffic + SBUF);
matmuls accumulate in fp32 PSUM. rel-err vs the f32 reference ~5e-3.
"""

import numpy as np

import concourse.bacc as bacc
import concourse.tile as tile
from concourse import mybir
from concourse.bass_utils import run_bass_kernel_spmd


def _install_ntff_hook():
    """Register the axon NTFF profile hook if the image's antenv lacks it.

    bass_utils needs antenv.axon_hooks.get_axon_ntff_profile_hook when
    BASS_TRACE=1; this image's antenv has no axon_hooks module, so build
    the ctypes hook against libaxon_pjrt.so directly. Fully guarded: any
    failure leaves tracing disabled but the kernel still runs.
    """
    import sys
    import types

    try:
        import antenv

        if "antenv.axon_hooks" in sys.modules:
            return
        import contextlib
        import ctypes

        so_path = "/opt/axon/libaxon_pjrt.so"
        lib = ctypes.CDLL(so_path)
        if not hasattr(lib, "axon_start_nrt_profile"):
            return
        lib.axon_start_nrt_profile.argtypes = [
            ctypes.POINTER(ctypes.c_int64),
            ctypes.c_size_t,
        ]
        lib.axon_start_nrt_profile.restype = ctypes.c_int64
        lib.axon_stop_nrt_profile.argtypes = [ctypes.c_char_p]
        lib.axon_stop_nrt_profile.restype = ctypes.c_int64

        @contextlib.contextmanager
        def _hook(output_dir, device_ids):
            import jax

            jax.devices()
            if device_ids:
                ids = (ctypes.c_int64 * len(device_ids))(*device_ids)
                rc = lib.axon_start_nrt_profile(ids, len(device_ids))
            else:
                rc = lib.axon_start_nrt_profile(None, 0)
            if rc != 0:
                raise RuntimeError(f"axon_start_nrt_profile rc={rc}")
            try:
                yield
            finally:
                n = lib.axon_stop_nrt_profile(str(output_dir).encode())
                if n <= 0:
                    print(f"ntff capture wrote {n} files", file=sys.stderr)

        mod = types.ModuleType("antenv.axon_hooks")
        mod.get_axon_ntff_profile_hook = lambda: _hook
        mod.set_axon_ntff_profile_hook = lambda h: None
        sys.modules["antenv.axon_hooks"] = mod
        antenv.axon_hooks = mod
    except Exception:
        pass


_install_ntff_hook()

F32 = mybir.dt.float32
F32R = mybir.dt.float32r
BF = mybir.dt.bfloat16
AF = mybir.ActivationFunctionType
OP = mybir.AluOpType

B, L, HID = 2, 2048, 2560
NH, NKV, D = 8, 4, 256
W = 512            # effective kv window (last W positions of the sequence)
CH = 512           # query rows per core
NCORES = 8
KT = HID // 128    # 20 contraction tiles for the projections
EPS = 1e-6
SOFTCAP = 50.0
SCALE = D ** -0.5
ROPE_BASE = 10000.0


def _build():
    nc = bacc.Bacc("TRN2", target_bir_lowering=False, debug=False,
                   num_devices=NCORES)
    xq = nc.dram_tensor("xq", [2, 128, KT // 2, CH], BF, kind="ExternalInput").ap()
    xkv = nc.dram_tensor("xkv", [2, 128, KT // 2, W], BF, kind="ExternalInput").ap()
    qw = nc.dram_tensor("qw", [NH, 128, KT, D], BF, kind="ExternalInput").ap()
    kw = nc.dram_tensor("kw", [128, KT, D], BF, kind="ExternalInput").ap()
    vw = nc.dram_tensor("vw", [128, KT, D], BF, kind="ExternalInput").ap()
    ow = nc.dram_tensor("ow", [HID // 128, 128, 16, 128], BF, kind="ExternalInput").ap()
    gkv_in = nc.dram_tensor("gkv_in", [128, 2 * W + NKV * D], BF).ap()
    gkv_out = nc.dram_tensor("gkv_out", [NKV, 128, 2 * W + NKV * D], BF).ap()
    # packed constant block, f32 columns:
    #   [0:CH) cq | [CH:2CH) sq | [2CH:2CH+W) ck | [2CH+W:2CH+2W) sk | 4 w1p
    NCONST = 2 * CH + 2 * W + 4
    cst = nc.dram_tensor("cst", [128, NCONST], F32, kind="ExternalInput").ap()
    yT = nc.dram_tensor("yT", [HID, CH], BF, kind="ExternalOutput").ap()

    with tile.TileContext(nc) as tc, \
            nc.allow_low_precision(reason='bf16 matmul operands'):
        with (
            tc.tile_pool(name="const", bufs=1) as pc,
            tc.tile_pool(name="px", bufs=1) as px,
            tc.tile_pool(name="pwk", bufs=1) as pwk,
            tc.tile_pool(name="pw", bufs=2) as pw,
            tc.tile_pool(name="pkv", bufs=1) as pkv,
            tc.tile_pool(name="pq", bufs=1) as pq,
            tc.tile_pool(name="ptmp", bufs=2) as ptmp,
            tc.tile_pool(name="prow", bufs=2) as prow,
            tc.tile_pool(name="pexp", bufs=2) as pexp,
            tc.tile_pool(name="pout", bufs=3) as pout,
            tc.tile_pool(name="pp", bufs=8, space="PSUM") as pp,
        ):
            # critical-path DMAs first: the k projection needs xkv + kw
            xkv_sb = px.tile([128, KT, W], BF, tag="xkv")
            for j in range(2):
                nc.sync.dma_start(out=xkv_sb[:, j * 10:(j + 1) * 10, :],
                                  in_=xkv[j])
            kw_sb = pwk.tile([128, KT, D], BF, tag="kw")
            nc.sync.dma_start(out=kw_sb, in_=kw)
            vw_sb = pwk.tile([128, KT, D], BF, tag="vw")
            nc.sync.dma_start(out=vw_sb, in_=vw)

            # constants (packed in one DMA; ones tiles via memset)
            cst_sb = pc.tile([128, NCONST], F32, tag="cst")
            nc.sync.dma_start(out=cst_sb, in_=cst)
            cq_sb = cst_sb[:, 0:CH]
            sq_sb = cst_sb[:, CH:2 * CH]
            ck_sb = cst_sb[:, 2 * CH:2 * CH + W]
            sk_sb = cst_sb[:, 2 * CH + W:2 * CH + 2 * W]
            w1p_sb = cst_sb[:, 2 * CH + 2 * W:]
            ones_colf = pc.tile([128, 1], F32, tag="onesc")
            nc.vector.memset(ones_colf, 1.0)
            ones_col = ones_colf.bitcast(F32R)
            ones_colb = pc.tile([128, 1], BF, tag="onescb")
            nc.vector.memset(ones_colb, 1.0)
            ones_rowf = pc.tile([1, 128], F32, tag="onesr")
            nc.vector.memset(ones_rowf, 1.0)
            ones_row = ones_rowf.bitcast(F32R)
            eps_sb = pc.tile([1, 1], F32, tag="eps")
            nc.vector.memset(eps_sb, EPS)

            def rms_rope(ps0, ps1, out0, out1, wcol0, wcol1, cos_sb, sin_sb,
                         nfree):
                """ps0/ps1: raw projected head-half tiles in PSUM.
                Writes rms-normalized, (1+w)-scaled, roped bf16 output to
                out0/out1."""
                ss_ps = pp.tile([1, nfree], F32, tag="bank")
                for i, ps in enumerate((ps0, ps1)):
                    sqt = ptmp.tile([128, nfree], F32R, tag="tA")
                    nc.scalar.square(sqt, ps)
                    nc.tensor.matmul(ss_ps, ones_col, sqt,
                                     start=(i == 0), stop=(i == 1))
                rs = prow.tile([1, nfree], F32R, tag="row")
                nc.scalar.activation(rs, ss_ps, AF.Sqrt, bias=eps_sb,
                                     scale=1.0 / D)
                rb_ps = pp.tile([128, nfree], F32, tag="bank")
                nc.tensor.matmul(rb_ps, ones_row, rs, start=True, stop=True)
                rb_sb = ptmp.tile([128, nfree], F32, tag="rbB")
                nc.vector.reciprocal_approx_fast(rb_sb, rb_ps)
                u0 = ptmp.tile([128, nfree], F32, tag="u0")
                u1 = ptmp.tile([128, nfree], F32, tag="u1")
                nc.vector.scalar_tensor_tensor(u0, ps0, wcol0, rb_sb,
                                               op0=OP.mult, op1=OP.mult)
                nc.vector.scalar_tensor_tensor(u1, ps1, wcol1, rb_sb,
                                               op0=OP.mult, op1=OP.mult)
                a = ptmp.tile([128, nfree], F32, tag="ra")
                bb = ptmp.tile([128, nfree], F32, tag="rb")
                nc.vector.tensor_mul(a, u0, cos_sb)
                nc.vector.tensor_mul(bb, u1, sin_sb)
                nc.vector.tensor_sub(out0, a, bb)
                a2 = ptmp.tile([128, nfree], F32, tag="ra")
                b2 = ptmp.tile([128, nfree], F32, tag="rb")
                nc.vector.tensor_mul(a2, u1, cos_sb)
                nc.vector.tensor_mul(b2, u0, sin_sb)
                nc.vector.tensor_add(out1, a2, b2)

            # ---- Phase 1: local kv projections (one kv head) + AllGather ----
            kps = [pp.tile([128, W], F32, tag="bank", name=f"kps{m}")
                   for m in range(2)]
            vps = [pp.tile([128, D], F32, tag="bank", name=f"vps{m}")
                   for m in range(4)]
            gate_mm = [None]
            for kbi in range(KT):
                for m in range(2):
                    mm = nc.tensor.matmul(
                        kps[m], kw_sb[:, kbi, m * 128:(m + 1) * 128],
                        xkv_sb[:, kbi, :],
                        start=(kbi == 0), stop=(kbi == KT - 1))
                    if gate_mm[0] is None:
                        gate_mm[0] = mm
            for kbi in range(KT):
                for m in range(4):
                    nc.tensor.matmul(
                        vps[m], xkv_sb[:, kbi, m * 128:(m + 1) * 128],
                        vw_sb[:, kbi, :],
                        start=(kbi == 0), stop=(kbi == KT - 1))
            khat_loc = pkv.tile([128, 2, W], BF, tag="khat_loc")
            rms_rope(kps[0], kps[1], khat_loc[:, 0, :], khat_loc[:, 1, :],
                     w1p_sb[:, 2:3], w1p_sb[:, 3:4], ck_sb, sk_sb, W)
            vloc = pkv.tile([128, NKV, D], BF, tag="vloc")
            for m in range(4):
                nc.vector.tensor_copy(vloc[:, m, :], vps[m])
            # pack + gather + unpack all ride the gpsimd queue so the sync /
            # scalar queues stay free for the q-projection stream.
            nc.gpsimd.dma_start(out=gkv_in[:, 0:2 * W], in_=khat_loc)
            nc.gpsimd.dma_start(out=gkv_in[:, 2 * W:], in_=vloc)
            nc.gpsimd.collective_compute(
                "AllGather", OP.bypass,
                replica_groups=[[0, 1, 2, 3], [4, 5, 6, 7]],
                ins=[gkv_in], outs=[gkv_out])
            khat = pkv.tile([128, 2 * NKV, W], BF, tag="khat")
            v_sb = pkv.tile([128, 4, NKV * D], BF, tag="v")
            for g in range(NKV):
                nc.sync.dma_start(out=khat[:, 2 * g:2 * g + 2, :],
                                  in_=gkv_out[g, :, 0:2 * W])
                nc.sync.dma_start(out=v_sb[:, :, g * D:(g + 1) * D],
                                  in_=gkv_out[g, :, 2 * W:])

            # ---- Phase 2: q projection (runs under the AllGather) ----
            xq_sb = px.tile([128, KT, CH], BF, tag="xq")
            gated_dmas = []
            for j in range(2):
                gated_dmas.append(nc.sync.dma_start(
                    out=xq_sb[:, j * 10:(j + 1) * 10, :], in_=xq[j]))
            qhat = pq.tile([128, 2 * NH, CH], BF, tag="qhat")
            for h in range(NH):
                qw_t = pw.tile([128, KT, D], BF, tag="qw")
                d = nc.sync.dma_start(out=qw_t, in_=qw[h])
                if h < 3:
                    gated_dmas.append(d)
                qps = [pp.tile([128, CH], F32, tag="bank",
                               name=f"qps{h}{m}") for m in range(2)]
                for kbi in range(KT):
                    for m in range(2):
                        nc.tensor.matmul(
                            qps[m], qw_t[:, kbi, m * 128:(m + 1) * 128],
                            xq_sb[:, kbi, :],
                            start=(kbi == 0), stop=(kbi == KT - 1))
                rms_rope(qps[0], qps[1], qhat[:, 2 * h, :],
                         qhat[:, 2 * h + 1, :],
                         w1p_sb[:, 0:1], w1p_sb[:, 1:2], cq_sb, sq_sb, CH)

            # ---- Phase 3: attention, software-pipelined over heads ----
            aoT = pq.tile([128, 2 * NH, CH], BF, tag="aoT")

            def attn_scores(h):
                g = h // 2
                sps = [pp.tile([128, CH], F32, tag="bank",
                               name=f"sps{h}{m}") for m in range(4)]
                exps = pexp.tile([128, 4, CH], BF, tag="exps")
                for mlk in range(4):
                    for dk in range(2):
                        nc.tensor.matmul(
                            sps[mlk],
                            khat[:, 2 * g + dk, mlk * 128:(mlk + 1) * 128],
                            qhat[:, 2 * h + dk, :],
                            start=(dk == 0), stop=(dk == 1))
                for mlk in range(4):
                    nc.scalar.activation(sps[mlk], sps[mlk], AF.Tanh,
                                         scale=SCALE / SOFTCAP)
                for mlk in range(4):
                    nc.scalar.activation(exps[:, mlk, :], sps[mlk], AF.Exp,
                                         scale=SOFTCAP)
                return exps

            def attn_tail(h, exps):
                g = h // 2
                dn_ps = pp.tile([1, CH], F32, tag="bank")
                for mlk in range(4):
                    nc.tensor.matmul(dn_ps, ones_colb, exps[:, mlk, :],
                                     start=(mlk == 0), stop=(mlk == 3))
                dn_sb = prow.tile([1, CH], F32R, tag="row2")
                nc.vector.tensor_copy(dn_sb, dn_ps)
                rb_ps = pp.tile([128, CH], F32, tag="bank")
                nc.tensor.matmul(rb_ps, ones_row, dn_sb,
                                 start=True, stop=True)
                rb_sb = ptmp.tile([128, CH], F32, tag="rbB")
                nc.vector.reciprocal_approx_fast(rb_sb, rb_ps)
                for dh in range(2):
                    ops = pp.tile([128, CH], F32, tag="bank")
                    base = g * D + dh * 128
                    for klk in range(4):
                        nc.tensor.matmul(
                            ops, v_sb[:, klk, base:base + 128],
                            exps[:, klk, :],
                            start=(klk == 0), stop=(klk == 3))
                    nc.vector.tensor_mul(aoT[:, 2 * h + dh, :], ops, rb_sb)

            prev = None
            for h in range(NH):
                e = attn_scores(h)
                if prev is not None:
                    attn_tail(*prev)
                prev = (h, e)
            attn_tail(*prev)

            # ---- Phase 4: o projection (outputs transposed: yT) ----
            for mp in range(HID // 128):
                owc = pw.tile([128, 16, 128], BF, tag="ow", bufs=4)
                nc.sync.dma_start(out=owc, in_=ow[mp])
                yps = pp.tile([128, CH], F32, tag="bank")
                for kk in range(16):
                    nc.tensor.matmul(yps, owc[:, kk, :], aoT[:, kk, :],
                                     start=(kk == 0), stop=(kk == 15))
                yst = pout.tile([128, CH], BF, tag="yst")
                if mp % 2 == 0:
                    nc.vector.tensor_copy(yst, yps)
                else:
                    nc.scalar.copy(yst, yps)
                nc.scalar.dma_start(out=yT[mp * 128:(mp + 1) * 128, :],
                                    in_=yst)

            # keep the startup HBM window clear for the kv-path loads: the
            # xq / early-qw streams only begin once the first k matmul (which
            # required kw + xkv chunk 0) has issued.
            from concourse.tile import add_dep_helper
            for d in gated_dmas:
                add_dep_helper(d.ins, gate_mm[0].ins, sync=False,
                               reason="delay q-path prefetch past kv startup")

    nc.compile()

    return nc


_NC_CACHE = {}


def _get_nc():
    if "nc" not in _NC_CACHE:
        _NC_CACHE["nc"] = _build()
    return _NC_CACHE["nc"]


def _rope_tables():
    inv_freq = 1.0 / (ROPE_BASE ** (np.arange(0, D, 2, dtype=np.float32) / D))
    t = np.arange(L, dtype=np.float32)
    freqs = np.outer(t, inv_freq)                     # (L, 128)
    return (np.ascontiguousarray(np.cos(freqs).T.astype(np.float32)),
            np.ascontiguousarray(np.sin(freqs).T.astype(np.float32)))


def kernel(x, mask, q_w, k_w, v_w, o_w, q_norm_w, k_norm_w):
    import ml_dtypes
    BF_NP = ml_dtypes.bfloat16

    x = np.asarray(x, dtype=np.float32)
    q_norm_w = np.asarray(q_norm_w, dtype=np.float32)
    k_norm_w = np.asarray(k_norm_w, dtype=np.float32)

    nc = _get_nc()

    qwb = np.asarray(q_w, dtype=np.float32).T.astype(BF_NP)   # (HID, 2048)
    kwb = np.asarray(k_w, dtype=np.float32).T.astype(BF_NP)   # (HID, 1024)
    vwb = np.asarray(v_w, dtype=np.float32).T.astype(BF_NP)
    owb = np.asarray(o_w, dtype=np.float32).T.astype(BF_NP)   # (2048, HID)

    # (NH, 128, KT, D)
    qw_p = np.ascontiguousarray(
        qwb.reshape(KT, 128, NH, D).transpose(2, 1, 0, 3))
    # per kv-head slices: (128, KT, D)
    kw_s = [np.ascontiguousarray(
        kwb[:, g * D:(g + 1) * D].reshape(KT, 128, D).transpose(1, 0, 2))
        for g in range(NKV)]
    vw_s = [np.ascontiguousarray(
        vwb[:, g * D:(g + 1) * D].reshape(KT, 128, D).transpose(1, 0, 2))
        for g in range(NKV)]
    # (20, 128, 16, 128)
    ow_p = np.ascontiguousarray(
        owb.reshape(16, 128, HID // 128, 128).transpose(2, 1, 0, 3))

    cosT, sinT = _rope_tables()                        # (128, L) each
    w1p = np.empty((128, 4), dtype=np.float32)
    w1p[:, 0] = 1.0 + q_norm_w[:128]
    w1p[:, 1] = 1.0 + q_norm_w[128:]
    w1p[:, 2] = 1.0 + k_norm_w[:128]
    w1p[:, 3] = 1.0 + k_norm_w[128:]

    def pretile_x(xt):
        # (HID, nfree) -> (2, 128, 10, nfree)
        nfree = xt.shape[1]
        return np.ascontiguousarray(
            xt.reshape(2, KT // 2, 128, nfree).transpose(0, 2, 1, 3))

    xb = x.astype(BF_NP)
    kv_lo = L - W
    xkv_b = [pretile_x(xb[b, kv_lo:, :].T) for b in range(B)]
    ckv = np.ascontiguousarray(cosT[:, kv_lo:])
    skv = np.ascontiguousarray(sinT[:, kv_lo:])

    in_maps = []
    for c in range(NCORES):
        b, j = divmod(c, 4)
        rows = slice(j * CH, (j + 1) * CH)
        cst = np.concatenate(
            [cosT[:, rows], sinT[:, rows], ckv, skv, w1p], axis=1)
        in_maps.append({
            "xq": pretile_x(xb[b, rows, :].T),
            "xkv": xkv_b[b],
            "qw": qw_p, "kw": kw_s[j], "vw": vw_s[j], "ow": ow_p,
            "cst": np.ascontiguousarray(cst),
        })

    res = run_bass_kernel_spmd(nc, in_maps, list(range(NCORES)))
    _NC_CACHE["last_res"] = res

    out = np.empty((B, L, HID), dtype=np.float32)
    for c in range(NCORES):
        b, j = divmod(c, 4)
        out[b, j * CH:(j + 1) * CH, :] = \
            res.results[c]["yT"].astype(np.float32).T
    return out
